# revision 12
# baseline (speedup 1.0000x reference)
"""AGREE group-recommendation forward pass on 8 TRN2 NeuronCores.

Data-parallel: B=1M batch sharded 131072/core; embedding tables and MLP
weights replicated per core. Per block of 2048 elements:
  1. indirect-DMA gather chain on gpsimd (group ids -> member triples ->
     user rows; item ids -> item rows), element-major in SBUF
  2. PE transposes to feature-major, bf16 matmul MLP pipeline
     (attention MLP -> softmax -> weighted member sum -> prediction MLP)
  3. PE transpose of y back to element order, DMA out
"""

import sys

sys.path.insert(0, "/opt/trn_rl_repo")

import numpy as np
import ml_dtypes

import bass_rust
import concourse.bass as bass
import concourse.mybir as mybir
import concourse.tile as tile_mod
from concourse.bass import IndirectOffsetOnAxis
from concourse.bass_utils import run_bass_kernel_spmd
from concourse.vector_clock import ScopedClock

NCORES = 8
B = 1048576
N = B // NCORES          # 131072 per core
BLK = 2048               # elements per block (128 partitions x 16)
JPB = BLK // 128         # 16 j-slots per partition per block
CH = 512                 # elements per matmul chunk (4 j-slots)
CPB = BLK // CH          # 4 chunks per block
NBLK = N // BLK          # 64
D = 32
NG = 50000
NU = 200000
NI = 100000

F32 = mybir.dt.float32
BF16 = mybir.dt.bfloat16
I32 = mybir.dt.int32
AF = mybir.ActivationFunctionType
MUL = mybir.AluOpType.mult
ADD = mybir.AluOpType.add

BENCH = {"trace": False}

# ---------------------------------------------------------------------------
# The neuronxcc in this container rejects instructions carrying >2 sync
# waits (CoreV3 setupSyncWait). Tile's end-of-context drain waits on the
# whole global clock in one instruction; split those waits across SP nops.
_MAXW = 1


def _patched_drain_and_barrier(self, tick_clock, wait_clock):
    probe = self.nc.sync.nop(nofuse=True, hint="drain_wait_split")
    wait_clock.add_sem_waits(probe.ins, ScopedClock({None: tick_clock.global_clock}))
    si = probe.ins.sync_info
    waits = list(si.on_wait) if si is not None else []
    ups = list(si.on_update) if si is not None else []
    probe.ins.sync_info = bass_rust.SyncInfo(on_wait=waits[:_MAXW], on_update=ups)
    for i in range(_MAXW, len(waits), _MAXW):
        n = self.nc.sync.nop(nofuse=True, hint="drain_wait_split")
        n.ins.sync_info = bass_rust.SyncInfo(
            on_wait=waits[i : i + _MAXW], on_update=[]
        )
    self.nc.sync.drain()
    self.nc.all_engine_barrier()
    assert self.sems is not None
    popped = self.nc._tile_sem_poison_stack.pop()
    assert popped is self._sem_poison
    self.nc.clear_and_free_semaphores(list(self.sems.allocated().values()))
    self.nc.all_engine_barrier()


tile_mod.TileContext._drain_and_barrier = _patched_drain_and_barrier


def _split_sync_waits(nc, max_waits=1):
    """Post-pass: no instruction may carry more than max_waits sem waits
    (neuronxcc setupSyncWait limit). Move excess waits onto preceding
    same-engine nops."""
    cnt = 0
    for f in nc.m.functions:
        for bb in f.blocks:
            out = []
            changed = False
            for inst in bb.instructions:
                si = inst.sync_info
                if si is not None and len(si.on_wait) > max_waits:
                    waits = list(si.on_wait)
                    ncarry = len(waits) - max_waits
                    for k in range(0, ncarry, max_waits):
                        cnt += 1
                        out.append(mybir.InstNoOp(
                            name=f"waitsplit-{cnt}",
                            engine=inst.engine,
                            bass_nofuse=True,
                            sync_info=mybir.SyncInfo(
                                on_wait=waits[k : k + max_waits], on_update=[]
                            ),
                        ))
                    inst.sync_info = mybir.SyncInfo(
                        on_wait=waits[ncarry:], on_update=list(si.on_update)
                    )
                    changed = True
                out.append(inst)
            if changed:
                bb.instructions = out
    return cnt
# ---------------------------------------------------------------------------


def build_program(n_elems=N, blk=BLK, split_waits=True):
    nblk = n_elems // blk
    jpb = blk // 128
    cpb = blk // CH

    nc = bass.Bass()
    gi_ext = nc.declare_dram_parameter("gi", [n_elems], I32, isOutput=False)
    it_ext = nc.declare_dram_parameter("it", [n_elems], I32, isOutput=False)
    me3_ext = nc.declare_dram_parameter("me3", [NG, 3 * D], F32, isOutput=False)
    ib_ext = nc.declare_dram_parameter("ib", [NI, D], F32, isOutput=False)
    w1_ext = nc.declare_dram_parameter("w1", [4 * D, 16], BF16, isOutput=False)
    b1_ext = nc.declare_dram_parameter("b1", [16], F32, isOutput=False)
    w2_ext = nc.declare_dram_parameter("w2", [16, 3], BF16, isOutput=False)
    b2_ext = nc.declare_dram_parameter("b2", [3], F32, isOutput=False)
    p1_ext = nc.declare_dram_parameter("p1", [3 * D, 8], BF16, isOutput=False)
    pb1_ext = nc.declare_dram_parameter("pb1", [8], F32, isOutput=False)
    p2_ext = nc.declare_dram_parameter("p2", [8, 16], BF16, isOutput=False)
    pb2_ext = nc.declare_dram_parameter("pb2", [4], F32, isOutput=False)
    idn_ext = nc.declare_dram_parameter("idn", [128, 128], F32, isOutput=False)
    out_ext = nc.declare_dram_parameter("out", [n_elems], F32, isOutput=True)
    stage = nc.dram_tensor("stage", [n_elems, 128], F32)

    with tile_mod.TileContext(nc) as tc:
        with (
            tc.tile_pool(name="const", bufs=1) as cp,
            tc.tile_pool(name="io", bufs=2) as io,
            tc.tile_pool(name="comp", bufs=2) as co,
            tc.tile_pool(name="ps", bufs=1, space="PSUM") as ps,
        ):
            w1me = cp.tile([96, 16], BF16)
            nc.sync.dma_start(out=w1me[:], in_=w1_ext[0:96, :])
            w1ie = cp.tile([96, 16], BF16)
            nc.sync.dma_start(out=w1ie[64:96, :], in_=w1_ext[96:128, :])
            b1sb = cp.tile([16, 1], F32)
            nc.sync.dma_start(out=b1sb[:], in_=b1_ext[:, None])
            w2sb = cp.tile([16, 3], BF16)
            nc.sync.dma_start(out=w2sb[:], in_=w2_ext[:])
            b2sb = cp.tile([3, 1], F32)
            nc.sync.dma_start(out=b2sb[:], in_=b2_ext[:, None])
            p1sb = cp.tile([96, 8], BF16)
            nc.sync.dma_start(out=p1sb[:], in_=p1_ext[:])
            pb1sb = cp.tile([8, 1], F32)
            nc.sync.dma_start(out=pb1sb[:], in_=pb1_ext[:, None])
            p2sb = cp.tile([8, 16], BF16)
            nc.sync.dma_start(out=p2sb[:], in_=p2_ext[:])
            pb2sb = cp.tile([4, 1], F32)
            nc.sync.dma_start(out=pb2sb[:], in_=pb2_ext[:, None])
            idnsb = cp.tile([128, 128], F32)
            nc.sync.dma_start(out=idnsb[:], in_=idn_ext[:])

            def gather_block(b):
                # straight-line phase A: indirect gathers -> DRAM stage
                gidx = io.tile([128, jpb], I32, tag="gidx")
                nc.sync.dma_start(
                    out=gidx[:],
                    in_=gi_ext[bass.ts(b, blk)].rearrange("(p j) -> p j", p=128),
                )
                iidx = io.tile([128, jpb], I32, tag="iidx")
                nc.sync.dma_start(
                    out=iidx[:],
                    in_=it_ext[bass.ts(b, blk)].rearrange("(p j) -> p j", p=128),
                )
                rec = io.tile([128, jpb * 128], F32, tag="rec")
                for k in range(jpb):
                    nc.gpsimd.indirect_dma_start(
                        out=rec[:, k * 128 : k * 128 + 96],
                        out_offset=None,
                        in_=me3_ext[:],
                        in_offset=IndirectOffsetOnAxis(
                            ap=gidx[:, k : k + 1], axis=0
                        ),
                    )
                    nc.gpsimd.indirect_dma_start(
                        out=rec[:, k * 128 + 96 : k * 128 + 128],
                        out_offset=None,
                        in_=ib_ext[:],
                        in_offset=IndirectOffsetOnAxis(
                            ap=iidx[:, k : k + 1], axis=0
                        ),
                    )
                nc.sync.dma_start(
                    out=stage[bass.ts(b, blk), :].rearrange(
                        "(p k) r -> p (k r)", p=128
                    ),
                    in_=rec[:],
                )

            def body(i):
                rec = io.tile([128, jpb * 128], F32, tag="recb")
                nc.sync.dma_start(
                    out=rec[:],
                    in_=stage[bass.ts(i, blk), :].rearrange(
                        "(p k) r -> p (k r)", p=128
                    ),
                )
                me = rec  # me record at [.., k*128 : k*128+96]
                iem = rec

                # element-major views: record[p, j, r] with j = 4*jj + c
                rec_v = rec[:].rearrange(
                    "p (jj c r) -> p jj c r", jj=jpb // 4, c=4, r=128
                )
                me_v = rec[:].rearrange(
                    "p (jj c m d) -> p jj c m d", jj=jpb // 4, c=4, m=4, d=D
                )[:, :, :, 0:3, :]
                ie_v = rec_v[:, :, :, 96:128]

                y_ps = ps.tile([4, CH], F32, tag="y_ps")
                for c in range(cpb):
                    xt_ps = ps.tile([96, CH], F32, tag="xt_ps")
                    iet_ps = ps.tile([32, CH], F32, tag="iet_ps")
                    for jj in range(4):
                        j = 4 * jj + c
                        nc.tensor.transpose(
                            out=xt_ps[:, 128 * jj : 128 * (jj + 1)],
                            in_=me[:, 128 * j : 128 * j + 96],
                            identity=idnsb[:],
                        )
                        nc.tensor.transpose(
                            out=iet_ps[:, 128 * jj : 128 * (jj + 1)],
                            in_=iem[:, 128 * j + 96 : 128 * j + 128],
                            identity=idnsb[:],
                        )
                    xt = co.tile([96, CH], BF16, tag="xt")
                    nc.vector.tensor_copy(out=xt[:], in_=xt_ps[:])
                    gg = co.tile([96, CH], BF16, tag="gg")
                    nc.scalar.activation(
                        out=gg[64:96, :], in_=iet_ps[:], func=AF.Copy
                    )

                    h_ps = ps.tile([16, CH], F32, tag="h_ps")
                    nc.tensor.matmul(
                        out=h_ps[:], lhsT=w1me[:], rhs=xt[:], start=True, stop=False
                    )
                    nc.tensor.matmul(
                        out=h_ps[:], lhsT=w1ie[64:96, :], rhs=gg[64:96, :],
                        start=False, stop=True,
                    )
                    hsb = co.tile([16, CH], BF16, tag="hsb")
                    nc.scalar.activation(
                        out=hsb[:], in_=h_ps[:], func=AF.Relu, bias=b1sb[:]
                    )

                    lg_ps = ps.tile([3, CH], F32, tag="lg_ps")
                    nc.tensor.matmul(
                        out=lg_ps[:], lhsT=w2sb[:], rhs=hsb[:],
                        start=True, stop=True,
                    )
                    ssb = co.tile([3, CH], F32, tag="ssb")
                    nc.scalar.activation(
                        out=ssb[:], in_=lg_ps[:], func=AF.Exp, bias=b2sb[:]
                    )

                    # transpose E = [e0,e1,e2] to element-major
                    st_ps = ps.tile([128, 12], F32, tag="st_ps")
                    for cc in range(4):
                        nc.tensor.transpose(
                            out=st_ps[:, 3 * cc : 3 * (cc + 1)],
                            in_=ssb[:, 128 * cc : 128 * (cc + 1)],
                            identity=idnsb[0:3, 0:3],
                        )
                    sts = co.tile([128, 12], F32, tag="sts")
                    nc.vector.tensor_copy(out=sts[:], in_=st_ps[:])
                    st_v = sts[:].rearrange("p (jj k) -> p jj k", k=3)
                    dsum = co.tile([128, 4], F32, tag="dsum")
                    nc.vector.tensor_reduce(
                        out=dsum[:], in_=st_v, axis=mybir.AxisListType.X, op=ADD
                    )
                    rsb = co.tile([128, 4], F32, tag="rsb")
                    nc.vector.reciprocal(out=rsb[:], in_=dsum[:])
                    e_t = st_v.unsqueeze(3).to_broadcast([128, 4, 3, D])
                    r_t = rsb[:].unsqueeze(2).to_broadcast([128, 4, D])

                    # g = (sum_m E_m * me_m) * r   (element-major)
                    prod = co.tile([128, 4 * 3 * D], F32, tag="prod")
                    prod_v = prod[:].rearrange("p (jj m d) -> p jj m d", m=3, d=D)
                    nc.vector.tensor_tensor(
                        out=prod_v, in0=me_v[:, :, c, :, :], in1=e_t, op=MUL
                    )
                    gu = co.tile([128, 4 * D], F32, tag="gu")
                    gu_v = gu[:].rearrange("p (jj d) -> p jj d", d=D)
                    prod_r = prod[:].rearrange("p (jj m d) -> p jj d m", m=3, d=D)
                    nc.vector.tensor_reduce(
                        out=gu_v, in_=prod_r, axis=mybir.AxisListType.X, op=ADD
                    )
                    g = co.tile([128, 4 * D], F32, tag="g")
                    g_v = g[:].rearrange("p (jj d) -> p jj d", d=D)
                    nc.vector.tensor_tensor(out=g_v, in0=gu_v, in1=r_t, op=MUL)
                    gie = co.tile([128, 4 * D], F32, tag="gie")
                    gie_v = gie[:].rearrange("p (jj d) -> p jj d", d=D)
                    nc.vector.tensor_tensor(
                        out=gie_v, in0=g_v, in1=ie_v[:, :, c, :], op=MUL
                    )

                    # feature-major [gie; g; ie] for the prediction MLP
                    giet_ps = ps.tile([32, CH], F32, tag="giet_ps")
                    gt_ps = ps.tile([32, CH], F32, tag="gt_ps")
                    for jj in range(4):
                        nc.tensor.transpose(
                            out=giet_ps[:, 128 * jj : 128 * (jj + 1)],
                            in_=gie[:, D * jj : D * (jj + 1)],
                            identity=idnsb[:],
                        )
                        nc.tensor.transpose(
                            out=gt_ps[:, 128 * jj : 128 * (jj + 1)],
                            in_=g[:, D * jj : D * (jj + 1)],
                            identity=idnsb[:],
                        )
                    nc.vector.tensor_copy(out=gg[0:32, :], in_=giet_ps[:])
                    nc.vector.tensor_copy(out=gg[32:64, :], in_=gt_ps[:])

                    h2_ps = ps.tile([8, CH], F32, tag="h_ps")
                    nc.tensor.matmul(
                        out=h2_ps[:], lhsT=p1sb[:], rhs=gg[:],
                        start=True, stop=True,
                    )
                    h2sb = co.tile([8, CH], BF16, tag="h2sb")
                    nc.scalar.activation(
                        out=h2sb[:], in_=h2_ps[:], func=AF.Relu, bias=pb1sb[:]
                    )
                    nc.tensor.matmul(
                        out=y_ps[:], lhsT=p2sb[:, 4 * c : 4 * (c + 1)], rhs=h2sb[:],
                        start=(c == 0), stop=(c == cpb - 1),
                    )

                ysb = co.tile([4, CH], F32, tag="ysb")
                nc.scalar.activation(
                    out=ysb[:], in_=y_ps[:], func=AF.Sigmoid, bias=pb2sb[:]
                )
                yt_ps = ps.tile([128, 16], F32, tag="st_ps")
                for cc in range(4):
                    nc.tensor.transpose(
                        out=yt_ps[:, 4 * cc : 4 * (cc + 1)],
                        in_=ysb[:, 128 * cc : 128 * (cc + 1)],
                        identity=idnsb[0:4, 0:4],
                    )
                yt = co.tile([128, 16], F32, tag="yt")
                nc.vector.tensor_copy(out=yt[:], in_=yt_ps[:])
                nc.sync.dma_start(
                    out=out_ext[bass.ts(i, blk)].rearrange("(p j) -> p j", p=128),
                    in_=yt[:],
                )

            for b in range(nblk):
                gather_block(b)
            if nblk == 1:
                body(0)
            else:
                with tc.For_i(0, nblk, 1) as i:
                    body(i)

    if split_waits:
        _split_sync_waits(nc)
    return nc


_prog_cache = {}


def _get_program(n_elems=N, blk=BLK):
    key = (n_elems, blk)
    if key not in _prog_cache:
        _prog_cache[key] = build_program(n_elems, blk)
    return _prog_cache[key]


def _p2_onehot(p2):
    out = np.zeros([8, 16], dtype=np.float32)
    for c in range(4):
        out[:, 4 * c + c] = p2.reshape(-1)
    return out


def make_in_maps(group_inputs, item_inputs, group_members, user_emb, item_emb,
                 att_w1, att_b1, att_w2, att_b2,
                 pred_w1, pred_b1, pred_w2, pred_b2, n_elems=N):
    bf16 = ml_dtypes.bfloat16
    gm = np.asarray(group_members, dtype=np.int64)
    ue = np.asarray(user_emb, dtype=np.float32)
    me3 = np.ascontiguousarray(ue[gm].reshape(NG, 3 * D))
    common = {
        "me3": me3,
        "ib": np.ascontiguousarray(np.asarray(item_emb, dtype=np.float32)),
        "w1": np.asarray(att_w1, dtype=np.float32).astype(bf16),
        "b1": np.asarray(att_b1, dtype=np.float32),
        "w2": np.asarray(att_w2, dtype=np.float32).astype(bf16),
        "b2": np.asarray(att_b2, dtype=np.float32),
        "p1": np.asarray(pred_w1, dtype=np.float32).astype(bf16),
        "pb1": np.asarray(pred_b1, dtype=np.float32),
        "p2": _p2_onehot(np.asarray(pred_w2, dtype=np.float32)).astype(bf16),
        "pb2": np.full([4], np.asarray(pred_b2, dtype=np.float32).reshape(-1)[0],
                       dtype=np.float32),
        "idn": np.eye(128, dtype=np.float32),
    }
    gi = np.asarray(group_inputs, dtype=np.int32)
    it = np.asarray(item_inputs, dtype=np.int32)
    in_maps = []
    for c in range(NCORES):
        m = dict(common)
        m["gi"] = np.ascontiguousarray(gi[c * n_elems : (c + 1) * n_elems])
        m["it"] = np.ascontiguousarray(it[c * n_elems : (c + 1) * n_elems])
        in_maps.append(m)
    return in_maps


def kernel(**inputs):
    nc = _get_program()
    in_maps = make_in_maps(**inputs)
    res = run_bass_kernel_spmd(
        nc, in_maps, core_ids=list(range(NCORES)), trace=BENCH.get("trace", False)
    )
    BENCH["last_result"] = res
    out = np.concatenate([res.results[c]["out"] for c in range(NCORES)])
    return out.reshape(B, 1).astype(np.float32)
